# revision 1
# baseline (speedup 1.0000x reference)
"""BitLinear (ternary-weight quantized linear) Trainium2 kernel.

Math (matches reference):
    delta  = mean(|W|) + 1e-5                    (global scalar)
    Wq     = clip(round(W/delta), -1, 1)         (ternary {-1,0,1})
    gamma  = max(|x|, axis=-1) + 1e-5            (per token row)
    k      = round(127*x/gamma)                  (integers in [-127,127])
    out    = (k @ Wq.T) * delta/127

Key facts exploited:
  * k and Wq are exact in bf16, so the 275-GFLOP matmul runs on the PE in
    bf16 with exact fp32 PSUM accumulation (|sum| <= 4096*127 < 2^24).
  * Rounding uses the fp32 magic-number trick (x + 1.5*2^23 - 1.5*2^23),
    which is round-to-nearest-even == jnp.round semantics.
  * W is passed host-side pre-transposed (W^T, layout [i, o]) so the
    contraction dim lands on SBUF partitions with no on-device W transpose.
  * x is quantized in natural layout (row max = free-axis reduce) and the
    small per-core x_q shard is transposed on the PE via identity matmuls.

Sharding: data-parallel over the 8192 token rows (1024 rows/core); W^T is
replicated.  delta needs a global abs-sum: each core reduces its 1/8 slice
of W and a device-side AllReduce combines them (single SPMD launch; the
delta pass overlaps the x-quantization phase).

Shipped kernel = build_v6_nc: PE-transpose phase X (keeps the PE warm and
busy during x staging), 1024-wide ternary W quant tiles, matmul loop in
m-groups of 2 with ping-pong PSUM bank sets so evacuation never stalls the
PE, early wq tags double-buffered across n-pair boundaries.  Measured on
hw: ~620 us/core steady-state body, rel err vs fp32 reference ~9e-5.
Earlier variants (v1..v5, fp8 DoubleRow experiment) kept below for
reference; fp8 fails accuracy (2.7e-2) because the k-quantization error
inflates ~5.5 sigma at the absmax over 33M outputs.
"""

import numpy as np
from contextlib import ExitStack

import concourse.bass as bass
import concourse.bacc as bacc
import concourse.tile as tile
import concourse.mybir as mybir
from concourse import masks
from concourse.bass_utils import run_bass_kernel_spmd

FP32 = mybir.dt.float32
BF16 = mybir.dt.bfloat16
ALU = mybir.AluOpType
AF = mybir.ActivationFunctionType
AX = mybir.AxisListType

N_CORES = 8
B, S, I = 4, 2048, 4096
O = 4096
R = B * S                    # 8192 token rows
RS = R // N_CORES            # 1024 rows per core
EPS = 1e-5
MAGIC = 12582912.0           # 1.5 * 2**23: fp32 round-to-nearest-even trick
KT = I // 128                # 32 contraction tiles
MT = RS // 128               # 8 row tiles per core
NT = O // 512                # 8 output-column blocks
W_SLICE = I // N_CORES       # 512 W^T rows per core for the delta pass


def _new_nc():
    return bacc.Bacc(
        "TRN2",
        target_bir_lowering=False,
        debug=False,
        enable_asserts=True,
        num_devices=N_CORES,
    )


def build_delta_nc(repeat=1):
    """Per-core partial abs-sums over a [512, 4096] slice of W.

    Emits the raw [128, 128] grid of 128-element chunk sums; the host does
    the remaining reduction in float64.  Any on-device fp32 tree past the
    chunk level injects ~2-5e-6 into delta, which flips ternary thresholds
    and (seed-dependent) costs up to ~1e-2 on the output absmax.
    """
    nc = _new_nc()
    ws = nc.dram_tensor("ws", [W_SLICE, I], FP32, kind="ExternalInput").ap()
    partial = nc.dram_tensor("partial", [128, 128], FP32, kind="ExternalOutput").ap()

    with tile.TileContext(nc) as tc, ExitStack() as ctx:
        pool = ctx.enter_context(tc.tile_pool(name="ld", bufs=2))
        spool = ctx.enter_context(tc.tile_pool(name="st", bufs=1))

        for _rep in range(repeat):
            acc = spool.tile([128, 128], FP32, tag="acc")
            for t in range(W_SLICE // 128):
                wtl = pool.tile([128, I], FP32, tag="wtl")
                nc.sync.dma_start(wtl, ws[128 * t : 128 * (t + 1), :])
                # chunked abs-sum: [128, 32, 128] --sum over last--> [128, 32]
                nc.vector.tensor_reduce(
                    acc[:, 32 * t : 32 * (t + 1)],
                    wtl.rearrange("p (c k) -> p c k", c=32),
                    axis=AX.X,
                    op=ALU.add,
                    apply_absolute_value=True,
                )
            nc.sync.dma_start(partial, acc)
    nc.compile()
    return nc


def build_main_nc(repeat=1, phases="xm"):
    """Main launch: quantize x shard + W^T, bf16 matmul, scale, store."""
    nc = _new_nc()
    xs = nc.dram_tensor("xs", [RS, I], FP32, kind="ExternalInput").ap()
    wt = nc.dram_tensor("wt", [I, O], FP32, kind="ExternalInput").ap()
    dsum = nc.dram_tensor("dsum", [128, 1], FP32, kind="ExternalInput").ap()
    out = nc.dram_tensor("out", [RS, O], FP32, kind="ExternalOutput").ap()

    with tile.TileContext(nc) as tc, ExitStack() as ctx:
        const_pool = ctx.enter_context(tc.tile_pool(name="const", bufs=1))
        xt_pool = ctx.enter_context(tc.tile_pool(name="xt", bufs=1))

        ident = const_pool.tile([128, 128], BF16)
        masks.make_identity(nc, ident)

        dsum_sb = const_pool.tile([128, 1], FP32)
        nc.sync.dma_start(dsum_sb, dsum)
        delta = const_pool.tile([128, 1], FP32)
        nc.vector.tensor_scalar(delta, dsum_sb, 1.0 / (I * O), EPS, ALU.mult, ALU.add)
        inv_delta = const_pool.tile([128, 1], FP32)
        nc.vector.reciprocal(inv_delta, delta)
        d127 = const_pool.tile([128, 1], FP32)
        nc.vector.tensor_scalar_mul(d127, delta, 1.0 / 127.0)

        # resident quantized-transposed activations: 32 x [128, 1024] bf16
        xt_tiles = [
            xt_pool.tile([128, RS], BF16, name=f"xt{k}", tag=f"xt{k}")
            for k in range(KT)
        ]

        if "x" not in phases:
            # timing-only variant: fill xt tiles so reads are legal
            for xt_t in xt_tiles:
                nc.vector.memset(xt_t, 1.0)

        for _rep in range(repeat):
            _main_body(nc, tc, xs, wt, out, xt_tiles, ident, inv_delta, d127,
                       phases=phases)
    nc.compile()
    return nc


def _main_body(nc, tc, xs, wt, out, xt_tiles, ident, inv_delta, d127, phases="xm"):
        # ---- Phase X: load, quantize, PE-transpose the x shard ----
        if "x" in phases:
          with ExitStack() as xctx:
            xpool = xctx.enter_context(tc.tile_pool(name="xload", bufs=2))
            tpool = xctx.enter_context(tc.tile_pool(name="xtmp", bufs=2))
            qpool = xctx.enter_context(tc.tile_pool(name="xq", bufs=2))
            gpool = xctx.enter_context(tc.tile_pool(name="gam", bufs=2))
            tpsum = xctx.enter_context(tc.tile_pool(name="tps", bufs=4, space="PSUM"))

            for m in range(MT):
                xtl = xpool.tile([128, I], FP32, tag="x")
                nc.sync.dma_start(xtl, xs[128 * m : 128 * (m + 1), :])
                gm = gpool.tile([128, 1], FP32, tag="gm")
                nc.vector.tensor_reduce(
                    gm, xtl, axis=AX.X, op=ALU.max, apply_absolute_value=True
                )
                gme = gpool.tile([128, 1], FP32, tag="gme")
                nc.vector.tensor_scalar_add(gme, gm, EPS)
                rec = gpool.tile([128, 1], FP32, tag="rec")
                nc.vector.reciprocal(rec, gme)
                sc = gpool.tile([128, 1], FP32, tag="sc")
                nc.vector.tensor_scalar_mul(sc, rec, 127.0)
                # t1 = x * (127/gamma) + MAGIC   (rounds to nearest even)
                t1 = tpool.tile([128, I], FP32, tag="t1")
                nc.vector.tensor_scalar(t1, xtl, sc, MAGIC, ALU.mult, ALU.add)
                # xq = t1 - MAGIC  -> integer k, exact in bf16
                xq = qpool.tile([128, I], BF16, tag="xq")
                nc.scalar.activation(xq, t1, AF.Copy, bias=-MAGIC, scale=1.0)
                # transpose each 128x128 block onto the resident xt tiles
                for k in range(KT):
                    pst = tpsum.tile([128, 128], BF16, tag="pst")
                    nc.tensor.transpose(pst, xq[:, 128 * k : 128 * (k + 1)], ident)
                    nc.scalar.copy(xt_tiles[k][:, 128 * m : 128 * (m + 1)], pst)

        # ---- Phase MM: stream W^T, quantize to ternary bf16, matmul ----
        if "m" in phases:
          with ExitStack() as mctx:
            wpool = mctx.enter_context(tc.tile_pool(name="wload", bufs=4))
            w1pool = mctx.enter_context(tc.tile_pool(name="w1", bufs=2))
            w2pool = mctx.enter_context(tc.tile_pool(name="w2", bufs=2))
            wqpool = mctx.enter_context(tc.tile_pool(name="wq", bufs=4))
            opool = mctx.enter_context(tc.tile_pool(name="ost", bufs=4))
            mpsum = mctx.enter_context(tc.tile_pool(name="mps", bufs=1, space="PSUM"))

            for n in range(NT):
                psums = [
                    mpsum.tile([128, 512], FP32, name=f"ps{m}", tag=f"ps{m}")
                    for m in range(MT)
                ]
                for k in range(KT):
                    wtl = wpool.tile([128, 512], FP32, tag="w")
                    nc.sync.dma_start(
                        wtl, wt[128 * k : 128 * (k + 1), 512 * n : 512 * (n + 1)]
                    )
                    # r = W/delta + MAGIC  (rounded to int by fp32 arithmetic)
                    w1 = w1pool.tile([128, 512], FP32, tag="w1")
                    nc.vector.tensor_scalar(w1, wtl, inv_delta, MAGIC, ALU.mult, ALU.add)
                    # clip to MAGIC +- 1  (== clip(round(W/delta), -1, 1))
                    w2 = w2pool.tile([128, 512], FP32, tag="w2")
                    nc.vector.tensor_scalar(w2, w1, MAGIC + 1.0, MAGIC - 1.0, ALU.min, ALU.max)
                    # subtract MAGIC -> ternary, cast bf16
                    wq = wqpool.tile([128, 512], BF16, tag="wq")
                    nc.scalar.activation(wq, w2, AF.Copy, bias=-MAGIC, scale=1.0)
                    for m in range(MT):
                        nc.tensor.matmul(
                            psums[m],
                            xt_tiles[k][:, 128 * m : 128 * (m + 1)],
                            wq,
                            start=(k == 0),
                            stop=(k == KT - 1),
                        )
                for m in range(MT):
                    ob = opool.tile([128, 512], FP32, tag="ob")
                    nc.scalar.activation(ob, psums[m], AF.Copy, bias=0.0, scale=d127)
                    nc.sync.dma_start(
                        out[128 * m : 128 * (m + 1), 512 * n : 512 * (n + 1)], ob
                    )


_NC_CACHE = {}


def _get_nc(name, repeat=1, phases="xm"):
    key = (name, repeat, phases)
    if key not in _NC_CACHE:
        if name == "delta":
            _NC_CACHE[key] = build_delta_nc(repeat=repeat)
        else:
            _NC_CACHE[key] = build_main_nc(repeat=repeat, phases=phases)
    return _NC_CACHE[key]


def kernel(x: np.ndarray, weight: np.ndarray) -> np.ndarray:
    """Two SPMD launches over 8 NeuronCores.

    Launch 1 computes per-core partial abs-sums of W (1/8 slice each); the
    host combines the 8x128 partials in float64 (the all-reduce step) so
    delta matches the fp32 reference to ~1e-7 -- a device-side fp32 tree
    costs ~5e-6 on delta, which flips enough ternary thresholds to push the
    output error from ~1e-4 to ~8e-3.  Launch 2 is the v6 body with the
    abs-sum broadcast as an input scalar.
    """
    x = np.asarray(x, dtype=np.float32)
    weight = np.asarray(weight, dtype=np.float32)
    core_ids = list(range(N_CORES))

    # host-side staging: W^T so the contraction dim is DMA-partition-major
    wtT = np.ascontiguousarray(weight.T)

    # ---- launch 1: per-core partial abs-sums over 1/8 of W ----
    nc_d = _get_nc("delta")
    in_maps_d = [
        {"ws": np.ascontiguousarray(wtT[c * W_SLICE : (c + 1) * W_SLICE, :])}
        for c in core_ids
    ]
    res_d = run_bass_kernel_spmd(nc_d, in_maps_d, core_ids)
    S_total = np.float64(0.0)
    for r in res_d.results:
        S_total += r["partial"].astype(np.float64).sum()
    dsum = np.full((128, 1), np.float32(S_total), dtype=np.float32)

    # ---- launch 2: v7 body (explicit ldweights reuse) with delta input ----
    key = ("v7", 1, "ixm")
    if key not in _NC_CACHE:
        _NC_CACHE[key] = build_v7_nc(delta_input=True)
    nc = _NC_CACHE[key]

    xf = np.ascontiguousarray(x.reshape(R, I))
    in_maps = [
        {
            "xs": np.ascontiguousarray(xf[c * RS : (c + 1) * RS, :]),
            "wt": wtT,
            "dsum": dsum,
        }
        for c in core_ids
    ]
    res = run_bass_kernel_spmd(nc, in_maps, core_ids)
    outs = [res.results[c]["out"] for c in core_ids]
    return np.concatenate(outs, axis=0).reshape(B, S, O)


if __name__ == "__main__":
    rng = np.random.default_rng(0)
    x = rng.standard_normal((B, S, I), dtype=np.float32)
    w = rng.standard_normal((O, I), dtype=np.float32)
    out = kernel(x, w)
    print("out shape", out.shape, "mean", out.mean(), "std", out.std())


# ======================= v2: single-launch kernel =======================

def build_single_nc(repeat=1, phases="dxm", gps_clip=True, fixed_wq=False):
    """Single launch: device-side delta (AllReduce), quantize, matmul.

    Improvements over v1:
      - delta abs-sum pass fused in (1/8 W slice per core + AllReduce of the
        per-partition totals), overlapped with the x phase.
      - x transposed on PE in fp32; the -MAGIC subtract + bf16 cast happen
        during the batched PSUM->SBUF evacuation (4 blocks per ACT op).
      - W quantized in [128,1024] tiles kept resident for a pair of n-blocks.
      - ternary clip runs on GPSIMD to unload the DVE.
    """
    nc = _new_nc()
    xs = nc.dram_tensor("xs", [RS, I], FP32, kind="ExternalInput").ap()
    wt = nc.dram_tensor("wt", [I, O], FP32, kind="ExternalInput").ap()
    ws = nc.dram_tensor("ws", [W_SLICE, I], FP32, kind="ExternalInput").ap()
    out = nc.dram_tensor("out", [RS, O], FP32, kind="ExternalOutput").ap()
    cc_in = nc.dram_tensor("cc_in", [128, 1], FP32).ap()
    cc_out = nc.dram_tensor("cc_out", [128, 1], FP32).ap()

    with tile.TileContext(nc) as tc, ExitStack() as ctx:
        const_pool = ctx.enter_context(tc.tile_pool(name="const", bufs=1))
        xt_pool = ctx.enter_context(tc.tile_pool(name="xt", bufs=1))

        ident = const_pool.tile([128, 128], FP32)
        masks.make_identity(nc, ident)
        ones = const_pool.tile([128, 128], FP32)
        nc.vector.memset(ones, 1.0)

        # single big tensor; xt column block k = xt_all[:, k*RS:(k+1)*RS]
        xt_all = xt_pool.tile([128, KT * RS], BF16)
        if "x" not in phases:
            nc.vector.memset(xt_all, 1.0)

        # ---- delta pass: abs-sum of this core's W slice, AllReduce ----
        if "d" in phases:
            with ExitStack() as dctx:
                dpool = dctx.enter_context(tc.tile_pool(name="dld", bufs=2))
                dsp = dctx.enter_context(tc.tile_pool(name="dst", bufs=1))
                dps = dctx.enter_context(tc.tile_pool(name="dps", bufs=1, space="PSUM"))
                acc = dsp.tile([128, 128], FP32)
                for t in range(W_SLICE // 128):
                    wtl0 = dpool.tile([128, I], FP32, tag="wtl0")
                    nc.sync.dma_start(wtl0, ws[128 * t : 128 * (t + 1), :])
                    nc.vector.tensor_reduce(
                        acc[:, 32 * t : 32 * (t + 1)],
                        wtl0.rearrange("p (c k) -> p c k", c=32),
                        axis=AX.X, op=ALU.add, apply_absolute_value=True,
                    )
                # broadcast-sum over partitions: ones[K,M].T @ acc[K,N] then
                # free-axis reduce -> per-partition core total
                bps = dps.tile([128, 128], FP32)
                nc.tensor.matmul(bps, ones, acc, start=True, stop=True)
                tot = dsp.tile([128, 1], FP32)
                nc.vector.tensor_reduce(tot, bps, axis=AX.X, op=ALU.add)
                nc.sync.dma_start(cc_in, tot)
                nc.gpsimd.collective_compute(
                    "AllReduce", ALU.add, replica_groups=[list(range(N_CORES))],
                    ins=[cc_in], outs=[cc_out],
                )
                gsum = dsp.tile([128, 1], FP32)
                nc.sync.dma_start(gsum, cc_out)
                delta = const_pool.tile([128, 1], FP32)
                nc.vector.tensor_scalar(delta, gsum, 1.0 / (I * O), EPS, ALU.mult, ALU.add)
                inv_delta = const_pool.tile([128, 1], FP32)
                nc.vector.reciprocal(inv_delta, delta)
                d127 = const_pool.tile([128, 1], FP32)
                nc.vector.tensor_scalar_mul(d127, delta, 1.0 / 127.0)
        else:
            delta = const_pool.tile([128, 1], FP32)
            nc.vector.memset(delta, 0.8)
            inv_delta = const_pool.tile([128, 1], FP32)
            nc.vector.reciprocal(inv_delta, delta)
            d127 = const_pool.tile([128, 1], FP32)
            nc.vector.tensor_scalar_mul(d127, delta, 1.0 / 127.0)

        for _rep in range(repeat):
            # ---- phase X ----
            if "x" in phases:
                with ExitStack() as xctx:
                    xpool = xctx.enter_context(tc.tile_pool(name="xload", bufs=3))
                    tpool = xctx.enter_context(tc.tile_pool(name="xtmp", bufs=2))
                    gpool = xctx.enter_context(tc.tile_pool(name="gam", bufs=2))
                    tpsum = xctx.enter_context(
                        tc.tile_pool(name="tps", bufs=3, space="PSUM"))

                    for m in range(MT):
                        xtl = xpool.tile([128, I], FP32, tag="x")
                        nc.sync.dma_start(xtl, xs[128 * m : 128 * (m + 1), :])
                        gm = gpool.tile([128, 1], FP32, tag="gm")
                        nc.vector.tensor_reduce(
                            gm, xtl, axis=AX.X, op=ALU.max, apply_absolute_value=True
                        )
                        gme = gpool.tile([128, 1], FP32, tag="gme")
                        nc.vector.tensor_scalar_add(gme, gm, EPS)
                        rec = gpool.tile([128, 1], FP32, tag="rec")
                        nc.vector.reciprocal(rec, gme)
                        sc = gpool.tile([128, 1], FP32, tag="sc")
                        nc.vector.tensor_scalar_mul(sc, rec, 127.0)
                        t1 = tpool.tile([128, I], FP32, tag="t1")
                        nc.vector.tensor_scalar(t1, xtl, sc, MAGIC, ALU.mult, ALU.add)
                        # fp32 PE transposes, 4 k-blocks batched per PSUM bank;
                        # evac does (-MAGIC) and bf16 cast in one ACT op
                        for kb in range(KT // 4):
                            pst = tpsum.tile([128, 512], FP32, tag="pst")
                            for j in range(4):
                                k = 4 * kb + j
                                nc.tensor.transpose(
                                    pst[:, 128 * j : 128 * (j + 1)],
                                    t1[:, 128 * k : 128 * (k + 1)],
                                    ident,
                                )
                            dst = xt_all.rearrange("p (k r) -> p k r", k=KT)[
                                :, 4 * kb : 4 * kb + 4, 128 * m : 128 * (m + 1)
                            ]
                            nc.scalar.activation(dst, pst, AF.Copy, bias=-MAGIC, scale=1.0)

            # ---- phase MM ----
            if "m" in phases:
                with ExitStack() as mctx:
                    wpool = mctx.enter_context(tc.tile_pool(name="wload", bufs=3))
                    w1pool = mctx.enter_context(tc.tile_pool(name="w1", bufs=2))
                    w2pool = mctx.enter_context(tc.tile_pool(name="w2", bufs=2))
                    wqpool = mctx.enter_context(tc.tile_pool(name="wq", bufs=1))
                    opool = mctx.enter_context(tc.tile_pool(name="ost", bufs=4))
                    mpsum = mctx.enter_context(
                        tc.tile_pool(name="mps", bufs=1, space="PSUM"))

                    for npair in range(NT // 2):
                        wqs = [
                            wqpool.tile([128, 1024], BF16, name=f"wq{k}", tag=f"wq{k}")
                            for k in range(KT)
                        ]
                        for k in range(1 if fixed_wq else KT):
                            wtl = wpool.tile([128, 1024], FP32, tag="w")
                            nc.sync.dma_start(
                                wtl,
                                wt[128 * k : 128 * (k + 1),
                                   1024 * npair : 1024 * (npair + 1)],
                            )
                            w1 = w1pool.tile([128, 1024], FP32, tag="w1")
                            nc.vector.tensor_scalar(
                                w1, wtl, inv_delta, MAGIC, ALU.mult, ALU.add)
                            w2 = w2pool.tile([128, 1024], FP32, tag="w2")
                            clip_eng = nc.gpsimd if gps_clip else nc.vector
                            clip_eng.tensor_scalar(
                                w2, w1, MAGIC + 1.0, MAGIC - 1.0, ALU.min, ALU.max)
                            nc.scalar.activation(
                                wqs[k], w2, AF.Copy, bias=-MAGIC, scale=1.0)
                        for half in range(2):
                            n = 2 * npair + half
                            psums = [
                                mpsum.tile([128, 512], FP32, name=f"ps{m}", tag=f"ps{m}")
                                for m in range(MT)
                            ]
                            for k in range(KT):
                                wq_use = wqs[0] if fixed_wq else wqs[k]
                                for m in range(MT):
                                    nc.tensor.matmul(
                                        psums[m],
                                        xt_all[:, k * RS + 128 * m : k * RS + 128 * (m + 1)],
                                        wq_use[:, 512 * half : 512 * (half + 1)],
                                        start=(k == 0),
                                        stop=(k == KT - 1),
                                    )
                            for m in range(MT):
                                ob = opool.tile([128, 512], FP32, tag="ob")
                                nc.scalar.activation(
                                    ob, psums[m], AF.Copy, bias=0.0, scale=d127)
                                nc.sync.dma_start(
                                    out[128 * m : 128 * (m + 1),
                                        512 * n : 512 * (n + 1)], ob,
                                )
    nc.compile()
    return nc


def kernel_v2(x: np.ndarray, weight: np.ndarray) -> np.ndarray:
    x = np.asarray(x, dtype=np.float32)
    weight = np.asarray(weight, dtype=np.float32)
    core_ids = list(range(N_CORES))
    wtT = np.ascontiguousarray(weight.T)
    key = ("single", 1, "dxm")
    if key not in _NC_CACHE:
        _NC_CACHE[key] = build_single_nc()
    nc = _NC_CACHE[key]
    xf = np.ascontiguousarray(x.reshape(R, I))
    in_maps = [
        {
            "xs": np.ascontiguousarray(xf[c * RS : (c + 1) * RS, :]),
            "wt": wtT,
            "ws": np.ascontiguousarray(wtT[c * W_SLICE : (c + 1) * W_SLICE, :]),
        }
        for c in core_ids
    ]
    res = run_bass_kernel_spmd(nc, in_maps, core_ids)
    outs = [res.results[c]["out"] for c in core_ids]
    return np.concatenate(outs, axis=0).reshape(B, S, O)


# ======================= v3: LDW-amortized + xbar transpose =======================

def build_v3_nc(repeat=1, phases="dxm", xbar_transpose=True):
    """v3: same math as v2 with PE-load reductions.

    - matmul loop restructured to m-groups of 4 so each LDWEIGHTS (stationary
      xt block) feeds 2 matmuls (the two 512-col halves of the wq pair);
      PSUM = 4 m-tags x 2 half-tags = 8 banks.
    - x transpose done by the DMA xbar (SBUF->SBUF, bf16) instead of the PE:
      quantize to bf16 via ACT, then 32 dma_start_transpose per row tile.
    """
    nc = _new_nc()
    xs = nc.dram_tensor("xs", [RS, I], FP32, kind="ExternalInput").ap()
    wt = nc.dram_tensor("wt", [I, O], FP32, kind="ExternalInput").ap()
    ws = nc.dram_tensor("ws", [W_SLICE, I], FP32, kind="ExternalInput").ap()
    out = nc.dram_tensor("out", [RS, O], FP32, kind="ExternalOutput").ap()
    cc_in = nc.dram_tensor("cc_in", [128, 1], FP32).ap()
    cc_out = nc.dram_tensor("cc_out", [128, 1], FP32).ap()

    with tile.TileContext(nc) as tc, ExitStack() as ctx:
        const_pool = ctx.enter_context(tc.tile_pool(name="const", bufs=1))
        xt_pool = ctx.enter_context(tc.tile_pool(name="xt", bufs=1))

        ident = const_pool.tile([128, 128], FP32)
        masks.make_identity(nc, ident)
        ones = const_pool.tile([128, 128], FP32)
        nc.vector.memset(ones, 1.0)

        xt_all = xt_pool.tile([128, KT * RS], BF16)
        if "x" not in phases:
            nc.vector.memset(xt_all, 1.0)

        if "d" in phases:
            with ExitStack() as dctx:
                dpool = dctx.enter_context(tc.tile_pool(name="dld", bufs=2))
                dsp = dctx.enter_context(tc.tile_pool(name="dst", bufs=1))
                dps = dctx.enter_context(tc.tile_pool(name="dps", bufs=1, space="PSUM"))
                acc = dsp.tile([128, 128], FP32)
                for t in range(W_SLICE // 128):
                    wtl0 = dpool.tile([128, I], FP32, tag="wtl0")
                    nc.sync.dma_start(wtl0, ws[128 * t : 128 * (t + 1), :])
                    nc.vector.tensor_reduce(
                        acc[:, 32 * t : 32 * (t + 1)],
                        wtl0.rearrange("p (c k) -> p c k", c=32),
                        axis=AX.X, op=ALU.add, apply_absolute_value=True,
                    )
                bps = dps.tile([128, 128], FP32)
                nc.tensor.matmul(bps, ones, acc, start=True, stop=True)
                tot = dsp.tile([128, 1], FP32)
                nc.vector.tensor_reduce(tot, bps, axis=AX.X, op=ALU.add)
                nc.sync.dma_start(cc_in, tot)
                nc.gpsimd.collective_compute(
                    "AllReduce", ALU.add, replica_groups=[list(range(N_CORES))],
                    ins=[cc_in], outs=[cc_out],
                )
                gsum = dsp.tile([128, 1], FP32)
                nc.sync.dma_start(gsum, cc_out)
                delta = const_pool.tile([128, 1], FP32)
                nc.vector.tensor_scalar(delta, gsum, 1.0 / (I * O), EPS, ALU.mult, ALU.add)
                inv_delta = const_pool.tile([128, 1], FP32)
                nc.vector.reciprocal(inv_delta, delta)
                d127 = const_pool.tile([128, 1], FP32)
                nc.vector.tensor_scalar_mul(d127, delta, 1.0 / 127.0)
        else:
            delta = const_pool.tile([128, 1], FP32)
            nc.vector.memset(delta, 0.8)
            inv_delta = const_pool.tile([128, 1], FP32)
            nc.vector.reciprocal(inv_delta, delta)
            d127 = const_pool.tile([128, 1], FP32)
            nc.vector.tensor_scalar_mul(d127, delta, 1.0 / 127.0)

        for _rep in range(repeat):
            if "x" in phases:
                with ExitStack() as xctx:
                    xpool = xctx.enter_context(tc.tile_pool(name="xload", bufs=3))
                    tpool = xctx.enter_context(tc.tile_pool(name="xtmp", bufs=2))
                    qpool = xctx.enter_context(tc.tile_pool(name="xq", bufs=2))
                    gpool = xctx.enter_context(tc.tile_pool(name="gam", bufs=2))
                    tpsum = xctx.enter_context(
                        tc.tile_pool(name="tps", bufs=3, space="PSUM"))
                    for m in range(MT):
                        xtl = xpool.tile([128, I], FP32, tag="x")
                        nc.sync.dma_start(xtl, xs[128 * m : 128 * (m + 1), :])
                        gm = gpool.tile([128, 1], FP32, tag="gm")
                        nc.vector.tensor_reduce(
                            gm, xtl, axis=AX.X, op=ALU.max, apply_absolute_value=True
                        )
                        gme = gpool.tile([128, 1], FP32, tag="gme")
                        nc.vector.tensor_scalar_add(gme, gm, EPS)
                        rec = gpool.tile([128, 1], FP32, tag="rec")
                        nc.vector.reciprocal(rec, gme)
                        sc = gpool.tile([128, 1], FP32, tag="sc")
                        nc.vector.tensor_scalar_mul(sc, rec, 127.0)
                        t1 = tpool.tile([128, I], FP32, tag="t1")
                        nc.vector.tensor_scalar(t1, xtl, sc, MAGIC, ALU.mult, ALU.add)
                        if xbar_transpose:
                            xq = qpool.tile([128, I], BF16, tag="xq")
                            nc.scalar.activation(xq, t1, AF.Copy, bias=-MAGIC, scale=1.0)
                            for k in range(KT):
                                nc.sync.dma_start_transpose(
                                    xt_all[:, k * RS + 128 * m : k * RS + 128 * (m + 1)],
                                    xq[:, 128 * k : 128 * (k + 1)],
                                )
                        else:
                            for kb in range(KT // 4):
                                pst = tpsum.tile([128, 512], FP32, tag="pst")
                                for j in range(4):
                                    k = 4 * kb + j
                                    nc.tensor.transpose(
                                        pst[:, 128 * j : 128 * (j + 1)],
                                        t1[:, 128 * k : 128 * (k + 1)],
                                        ident,
                                    )
                                dst = xt_all.rearrange("p (k r) -> p k r", k=KT)[
                                    :, 4 * kb : 4 * kb + 4, 128 * m : 128 * (m + 1)
                                ]
                                nc.scalar.activation(dst, pst, AF.Copy, bias=-MAGIC, scale=1.0)

            if "m" in phases:
                with ExitStack() as mctx:
                    wpool = mctx.enter_context(tc.tile_pool(name="wload", bufs=3))
                    w1pool = mctx.enter_context(tc.tile_pool(name="w1", bufs=2))
                    w2pool = mctx.enter_context(tc.tile_pool(name="w2", bufs=2))
                    wqpool = mctx.enter_context(tc.tile_pool(name="wq", bufs=1))
                    opool = mctx.enter_context(tc.tile_pool(name="ost", bufs=4))
                    mpsum = mctx.enter_context(
                        tc.tile_pool(name="mps", bufs=1, space="PSUM"))

                    for npair in range(NT // 2):
                        wqs = [
                            wqpool.tile([128, 1024], BF16, name=f"wq{k}", tag=f"wq{k}")
                            for k in range(KT)
                        ]
                        for k in range(KT):
                            wtl = wpool.tile([128, 1024], FP32, tag="w")
                            nc.sync.dma_start(
                                wtl,
                                wt[128 * k : 128 * (k + 1),
                                   1024 * npair : 1024 * (npair + 1)],
                            )
                            w1 = w1pool.tile([128, 1024], FP32, tag="w1")
                            nc.vector.tensor_scalar(
                                w1, wtl, inv_delta, MAGIC, ALU.mult, ALU.add)
                            w2 = w2pool.tile([128, 1024], FP32, tag="w2")
                            nc.vector.tensor_scalar(
                                w2, w1, MAGIC + 1.0, MAGIC - 1.0, ALU.min, ALU.max)
                            nc.scalar.activation(
                                wqs[k], w2, AF.Copy, bias=-MAGIC, scale=1.0)
                        # m-groups of 4: each stationary xt block feeds both halves
                        for mg in range(MT // 4):
                            psums = {}
                            for mi in range(4):
                                for half in range(2):
                                    psums[(mi, half)] = mpsum.tile(
                                        [128, 512], FP32,
                                        name=f"ps{mi}_{half}", tag=f"ps{mi}_{half}")
                            for k in range(KT):
                                for mi in range(4):
                                    m = 4 * mg + mi
                                    for half in range(2):
                                        nc.tensor.matmul(
                                            psums[(mi, half)],
                                            xt_all[:, k * RS + 128 * m : k * RS + 128 * (m + 1)],
                                            wqs[k][:, 512 * half : 512 * (half + 1)],
                                            start=(k == 0),
                                            stop=(k == KT - 1),
                                        )
                            for mi in range(4):
                                m = 4 * mg + mi
                                for half in range(2):
                                    n = 2 * npair + half
                                    ob = opool.tile([128, 512], FP32, tag="ob")
                                    nc.scalar.activation(
                                        ob, psums[(mi, half)], AF.Copy, bias=0.0, scale=d127)
                                    nc.sync.dma_start(
                                        out[128 * m : 128 * (m + 1),
                                            512 * n : 512 * (n + 1)], ob,
                                    )
    nc.compile()
    return nc


# ======================= v4: fp8 DoubleRow =======================

FP8 = mybir.dt.float8e4


def build_v4_nc(repeat=1, phases="dxm", xbar_transpose=True):
    """fp8e4m3 DoubleRow matmul: 2 k-planes per MM, 2x PE ALU rate.

    - xt_all fp8, m-major layout [128, MT, KT, 128]: lhsT AP for (m, k-pair)
      is [128, 2, 128]; x ints in [-127,127] round to fp8 (~0.5% avg err,
      ~2.6e-3 on the output absmax scale).
    - wq_all fp8 ternary (exact), k-major [128, KT*2048]; 2048-wide quant
      tiles let one stationary load feed 4 n-halves (LDW count /8).
    """
    nc = _new_nc()
    xs = nc.dram_tensor("xs", [RS, I], FP32, kind="ExternalInput").ap()
    wt = nc.dram_tensor("wt", [I, O], FP32, kind="ExternalInput").ap()
    ws = nc.dram_tensor("ws", [W_SLICE, I], FP32, kind="ExternalInput").ap()
    out = nc.dram_tensor("out", [RS, O], FP32, kind="ExternalOutput").ap()
    cc_in = nc.dram_tensor("cc_in", [128, 1], FP32).ap()
    cc_out = nc.dram_tensor("cc_out", [128, 1], FP32).ap()

    with tile.TileContext(nc) as tc, ExitStack() as ctx:
        const_pool = ctx.enter_context(tc.tile_pool(name="const", bufs=1))
        xt_pool = ctx.enter_context(tc.tile_pool(name="xt", bufs=1))

        ident = const_pool.tile([128, 128], FP32)
        masks.make_identity(nc, ident)
        ones = const_pool.tile([128, 128], FP32)
        nc.vector.memset(ones, 1.0)

        xt_all = xt_pool.tile([128, MT * KT * 128], FP8)   # m-major
        if "x" not in phases:
            nc.vector.memset(xt_all, 1.0)
        xt4 = xt_all.rearrange("p (m k r) -> p m k r", m=MT, k=KT)

        if "d" in phases:
            with ExitStack() as dctx:
                dpool = dctx.enter_context(tc.tile_pool(name="dld", bufs=2))
                dsp = dctx.enter_context(tc.tile_pool(name="dst", bufs=1))
                dps = dctx.enter_context(tc.tile_pool(name="dps", bufs=1, space="PSUM"))
                acc = dsp.tile([128, 128], FP32)
                for t in range(W_SLICE // 128):
                    wtl0 = dpool.tile([128, I], FP32, tag="wtl0")
                    nc.sync.dma_start(wtl0, ws[128 * t : 128 * (t + 1), :])
                    nc.vector.tensor_reduce(
                        acc[:, 32 * t : 32 * (t + 1)],
                        wtl0.rearrange("p (c k) -> p c k", c=32),
                        axis=AX.X, op=ALU.add, apply_absolute_value=True,
                    )
                bps = dps.tile([128, 128], FP32)
                nc.tensor.matmul(bps, ones, acc, start=True, stop=True)
                tot = dsp.tile([128, 1], FP32)
                nc.vector.tensor_reduce(tot, bps, axis=AX.X, op=ALU.add)
                nc.sync.dma_start(cc_in, tot)
                nc.gpsimd.collective_compute(
                    "AllReduce", ALU.add, replica_groups=[list(range(N_CORES))],
                    ins=[cc_in], outs=[cc_out],
                )
                gsum = dsp.tile([128, 1], FP32)
                nc.sync.dma_start(gsum, cc_out)
                delta = const_pool.tile([128, 1], FP32)
                nc.vector.tensor_scalar(delta, gsum, 1.0 / (I * O), EPS, ALU.mult, ALU.add)
                inv_delta = const_pool.tile([128, 1], FP32)
                nc.vector.reciprocal(inv_delta, delta)
                d127 = const_pool.tile([128, 1], FP32)
                nc.vector.tensor_scalar_mul(d127, delta, 1.0 / 127.0)
        else:
            delta = const_pool.tile([128, 1], FP32)
            nc.vector.memset(delta, 0.8)
            inv_delta = const_pool.tile([128, 1], FP32)
            nc.vector.reciprocal(inv_delta, delta)
            d127 = const_pool.tile([128, 1], FP32)
            nc.vector.tensor_scalar_mul(d127, delta, 1.0 / 127.0)

        for _rep in range(repeat):
            if "x" in phases:
                with ExitStack() as xctx:
                    xpool = xctx.enter_context(tc.tile_pool(name="xload", bufs=3))
                    tpool = xctx.enter_context(tc.tile_pool(name="xtmp", bufs=2))
                    qpool = xctx.enter_context(tc.tile_pool(name="xq", bufs=2))
                    spool = xctx.enter_context(tc.tile_pool(name="xstg", bufs=2))
                    gpool = xctx.enter_context(tc.tile_pool(name="gam", bufs=2))
                    tpsum = xctx.enter_context(
                        tc.tile_pool(name="tps", bufs=3, space="PSUM"))
                    for m in range(MT):
                        xtl = xpool.tile([128, I], FP32, tag="x")
                        nc.sync.dma_start(xtl, xs[128 * m : 128 * (m + 1), :])
                        gm = gpool.tile([128, 1], FP32, tag="gm")
                        nc.vector.tensor_reduce(
                            gm, xtl, axis=AX.X, op=ALU.max, apply_absolute_value=True
                        )
                        gme = gpool.tile([128, 1], FP32, tag="gme")
                        nc.vector.tensor_scalar_add(gme, gm, EPS)
                        rec = gpool.tile([128, 1], FP32, tag="rec")
                        nc.vector.reciprocal(rec, gme)
                        sc = gpool.tile([128, 1], FP32, tag="sc")
                        nc.vector.tensor_scalar_mul(sc, rec, 127.0)
                        t1 = tpool.tile([128, I], FP32, tag="t1")
                        nc.vector.tensor_scalar(t1, xtl, sc, MAGIC, ALU.mult, ALU.add)
                        if xbar_transpose:
                            xq = qpool.tile([128, I], BF16, tag="xq")
                            nc.scalar.activation(xq, t1, AF.Copy, bias=-MAGIC, scale=1.0)
                            stg = spool.tile([128, KT * 128], BF16, tag="stg")
                            for k in range(KT):
                                nc.sync.dma_start_transpose(
                                    stg[:, 128 * k : 128 * (k + 1)],
                                    xq[:, 128 * k : 128 * (k + 1)],
                                )
                            nc.vector.tensor_copy(
                                xt_all[:, m * KT * 128 : (m + 1) * KT * 128], stg)
                        else:
                            for kb in range(KT // 4):
                                pst = tpsum.tile([128, 512], FP32, tag="pst")
                                for j in range(4):
                                    k = 4 * kb + j
                                    nc.tensor.transpose(
                                        pst[:, 128 * j : 128 * (j + 1)],
                                        t1[:, 128 * k : 128 * (k + 1)],
                                        ident,
                                    )
                                nc.scalar.activation(
                                    xt4[:, m, 4 * kb : 4 * kb + 4, :],
                                    pst.rearrange("p (a b) -> p a b", a=4),
                                    AF.Copy, bias=-MAGIC, scale=1.0)

            if "m" in phases:
                with ExitStack() as mctx:
                    wpool = mctx.enter_context(tc.tile_pool(name="wload", bufs=3))
                    w1pool = mctx.enter_context(tc.tile_pool(name="w1", bufs=2))
                    w2pool = mctx.enter_context(tc.tile_pool(name="w2", bufs=2))
                    wqpool = mctx.enter_context(tc.tile_pool(name="wq", bufs=1))
                    opool = mctx.enter_context(tc.tile_pool(name="ost", bufs=4))
                    mpsum = mctx.enter_context(
                        tc.tile_pool(name="mps", bufs=1, space="PSUM"))

                    for nq in range(NT // 4):
                        wq_all = wqpool.tile([128, KT * 2048], FP8, tag="wqa")
                        wq3 = wq_all.rearrange("p (k w) -> p k w", k=KT)
                        for k in range(KT):
                            wtl = wpool.tile([128, 2048], FP32, tag="w")
                            nc.sync.dma_start(
                                wtl,
                                wt[128 * k : 128 * (k + 1),
                                   2048 * nq : 2048 * (nq + 1)],
                            )
                            w1 = w1pool.tile([128, 2048], FP32, tag="w1")
                            nc.vector.tensor_scalar(
                                w1, wtl, inv_delta, MAGIC, ALU.mult, ALU.add)
                            w2 = w2pool.tile([128, 2048], FP32, tag="w2")
                            nc.vector.tensor_scalar(
                                w2, w1, MAGIC + 1.0, MAGIC - 1.0, ALU.min, ALU.max)
                            nc.scalar.activation(
                                wq3[:, k, :], w2, AF.Copy, bias=-MAGIC, scale=1.0)
                        for mg in range(MT // 2):
                            psums = {}
                            for mi in range(2):
                                for half in range(4):
                                    psums[(mi, half)] = mpsum.tile(
                                        [128, 512], FP32,
                                        name=f"ps{mi}_{half}", tag=f"ps{mi}_{half}")
                            for t in range(KT // 2):
                                for mi in range(2):
                                    m = 2 * mg + mi
                                    for half in range(4):
                                        nc.tensor.matmul(
                                            psums[(mi, half)],
                                            xt4[:, m, 2 * t : 2 * t + 2, :],
                                            wq3[:, 2 * t : 2 * t + 2,
                                                512 * half : 512 * (half + 1)],
                                            start=(t == 0),
                                            stop=(t == KT // 2 - 1),
                                            perf_mode=mybir.MatmulPerfMode.DoubleRow,
                                        )
                            for mi in range(2):
                                m = 2 * mg + mi
                                for half in range(4):
                                    n = 4 * nq + half
                                    ob = opool.tile([128, 512], FP32, tag="ob")
                                    nc.scalar.activation(
                                        ob, psums[(mi, half)], AF.Copy, bias=0.0, scale=d127)
                                    nc.sync.dma_start(
                                        out[128 * m : 128 * (m + 1),
                                            512 * n : 512 * (n + 1)], ob,
                                    )
    nc.compile()
    return nc


# ======================= v6: v5 + psum ping-pong =======================

def build_v6_nc(repeat=1, phases="dxm", delta_input=False):
    """PE-transpose phase X + LDW-amortized MM with ping-pong PSUM sets.

    MM phase: m-groups of 2, psum tag set alternates (A/B) between groups so
    the 4 banks of the previous group evacuate while the PE fills the other 4.
    """
    nc = _new_nc()
    xs = nc.dram_tensor("xs", [RS, I], FP32, kind="ExternalInput").ap()
    wt = nc.dram_tensor("wt", [I, O], FP32, kind="ExternalInput").ap()
    if delta_input:
        dsum = nc.dram_tensor("dsum", [128, 1], FP32, kind="ExternalInput").ap()
    else:
        ws = nc.dram_tensor("ws", [W_SLICE, I], FP32, kind="ExternalInput").ap()
        cc_in = nc.dram_tensor("cc_in", [128, 1], FP32).ap()
        cc_out = nc.dram_tensor("cc_out", [128, 1], FP32).ap()
    out = nc.dram_tensor("out", [RS, O], FP32, kind="ExternalOutput").ap()

    with tile.TileContext(nc) as tc, ExitStack() as ctx:
        const_pool = ctx.enter_context(tc.tile_pool(name="const", bufs=1))
        xt_pool = ctx.enter_context(tc.tile_pool(name="xt", bufs=1))

        ident = const_pool.tile([128, 128], FP32)
        masks.make_identity(nc, ident)
        ones = const_pool.tile([128, 128], FP32)
        nc.vector.memset(ones, 1.0)

        xt_all = xt_pool.tile([128, KT * RS], BF16)
        if "x" not in phases:
            nc.vector.memset(xt_all, 1.0)

        if delta_input:
            dsum_sb = const_pool.tile([128, 1], FP32)
            nc.sync.dma_start(dsum_sb, dsum)
            delta = const_pool.tile([128, 1], FP32)
            nc.vector.tensor_scalar(delta, dsum_sb, 1.0 / (I * O), EPS, ALU.mult, ALU.add)
            inv_delta = const_pool.tile([128, 1], FP32)
            nc.vector.reciprocal(inv_delta, delta)
            d127 = const_pool.tile([128, 1], FP32)
            nc.vector.tensor_scalar_mul(d127, delta, 1.0 / 127.0)
        elif "d" in phases:
            with ExitStack() as dctx:
                dpool = dctx.enter_context(tc.tile_pool(name="dld", bufs=2))
                dsp = dctx.enter_context(tc.tile_pool(name="dst", bufs=1))
                dps = dctx.enter_context(tc.tile_pool(name="dps", bufs=1, space="PSUM"))
                acc = dsp.tile([128, 128], FP32)
                for t in range(W_SLICE // 128):
                    wtl0 = dpool.tile([128, I], FP32, tag="wtl0")
                    nc.sync.dma_start(wtl0, ws[128 * t : 128 * (t + 1), :])
                    nc.vector.tensor_reduce(
                        acc[:, 32 * t : 32 * (t + 1)],
                        wtl0.rearrange("p (c k) -> p c k", c=32),
                        axis=AX.X, op=ALU.add, apply_absolute_value=True,
                    )
                bps = dps.tile([128, 128], FP32)
                nc.tensor.matmul(bps, ones, acc, start=True, stop=True)
                tot = dsp.tile([128, 1], FP32)
                nc.vector.tensor_reduce(tot, bps, axis=AX.X, op=ALU.add)
                nc.sync.dma_start(cc_in, tot)
                nc.gpsimd.collective_compute(
                    "AllReduce", ALU.add, replica_groups=[list(range(N_CORES))],
                    ins=[cc_in], outs=[cc_out],
                )
                gsum = dsp.tile([128, 1], FP32)
                nc.sync.dma_start(gsum, cc_out)
                delta = const_pool.tile([128, 1], FP32)
                nc.vector.tensor_scalar(delta, gsum, 1.0 / (I * O), EPS, ALU.mult, ALU.add)
                inv_delta = const_pool.tile([128, 1], FP32)
                nc.vector.reciprocal(inv_delta, delta)
                d127 = const_pool.tile([128, 1], FP32)
                nc.vector.tensor_scalar_mul(d127, delta, 1.0 / 127.0)
        else:
            delta = const_pool.tile([128, 1], FP32)
            nc.vector.memset(delta, 0.8)
            inv_delta = const_pool.tile([128, 1], FP32)
            nc.vector.reciprocal(inv_delta, delta)
            d127 = const_pool.tile([128, 1], FP32)
            nc.vector.tensor_scalar_mul(d127, delta, 1.0 / 127.0)

        for _rep in range(repeat):
            if "x" in phases:
                with ExitStack() as xctx:
                    xpool = xctx.enter_context(tc.tile_pool(name="xload", bufs=3))
                    tpool = xctx.enter_context(tc.tile_pool(name="xtmp", bufs=2))
                    gpool = xctx.enter_context(tc.tile_pool(name="gam", bufs=2))
                    tpsum = xctx.enter_context(
                        tc.tile_pool(name="tps", bufs=3, space="PSUM"))
                    for m in range(MT):
                        xtl = xpool.tile([128, I], FP32, tag="x")
                        nc.sync.dma_start(xtl, xs[128 * m : 128 * (m + 1), :])
                        gm = gpool.tile([128, 1], FP32, tag="gm")
                        nc.vector.tensor_reduce(
                            gm, xtl, axis=AX.X, op=ALU.max, apply_absolute_value=True
                        )
                        gme = gpool.tile([128, 1], FP32, tag="gme")
                        nc.vector.tensor_scalar_add(gme, gm, EPS)
                        rec = gpool.tile([128, 1], FP32, tag="rec")
                        nc.vector.reciprocal(rec, gme)
                        sc = gpool.tile([128, 1], FP32, tag="sc")
                        nc.vector.tensor_scalar_mul(sc, rec, 127.0)
                        t1 = tpool.tile([128, I], FP32, tag="t1")
                        nc.vector.tensor_scalar(t1, xtl, sc, MAGIC, ALU.mult, ALU.add)
                        for kb in range(KT // 4):
                            pst = tpsum.tile([128, 512], FP32, tag="pst")
                            for j in range(4):
                                k = 4 * kb + j
                                nc.tensor.transpose(
                                    pst[:, 128 * j : 128 * (j + 1)],
                                    t1[:, 128 * k : 128 * (k + 1)],
                                    ident,
                                )
                            dst = xt_all.rearrange("p (k r) -> p k r", k=KT)[
                                :, 4 * kb : 4 * kb + 4, 128 * m : 128 * (m + 1)
                            ]
                            nc.scalar.activation(dst, pst, AF.Copy, bias=-MAGIC, scale=1.0)

            if "m" in phases:
                with ExitStack() as mctx:
                    wpool = mctx.enter_context(tc.tile_pool(name="wload", bufs=3))
                    w1pool = mctx.enter_context(tc.tile_pool(name="w1", bufs=2))
                    w2pool = mctx.enter_context(tc.tile_pool(name="w2", bufs=2))
                    wqpool = mctx.enter_context(tc.tile_pool(name="wq", bufs=1))
                    opool = mctx.enter_context(tc.tile_pool(name="ost", bufs=4))
                    mpsum = mctx.enter_context(
                        tc.tile_pool(name="mps", bufs=1, space="PSUM"))

                    mg_idx = 0
                    for npair in range(NT // 2):
                        wqs = [
                            wqpool.tile([128, 1024], BF16, name=f"wq{k}", tag=f"wq{k}",
                                        bufs=2 if k < 6 else 1)
                            for k in range(KT)
                        ]
                        for k in range(KT):
